# revision 35
# baseline (speedup 1.0000x reference)
"""CWCT (class-wise whitening/coloring transform) for Trainium2, 8 NeuronCores.

Strategy
--------
Pixels are counting-sorted by segment label on the host (pure data
movement); each label's pixel range is split contiguously across the 8
cores, zero-padded to a fixed per-(core,label) capacity C.

Device phase 1 (per core): for every label, accumulate the raw second
moment S_l = sum_p x_p x_p^T and the channel sums over that core's pixel
shard, for content and style, as grouped 128-pixel-contraction matmuls
into PSUM (bf16 operands, f32 accumulate). A ones-column appended to the
gathered arrays yields the channel sums for free in the same matmuls.

Host middle: all-reduce the (tiny) per-core partial moments, form
covariances, Cholesky factors, inv_Lc via triangular solve (float64),
combined transform T_l = Ls @ inv_Lc and bias b_l = mu_s - T_l mu_c.
Invalid labels get T = I, b = 0 (and are restored exactly from the
original content on the host at assembly time).

Device phase 2 (per core): colored = T_l @ x + b_l applied per label with
T stationary in the PE array, streaming channel-major pixel blocks.

Host end: scatter the colored pixels back to the original pixel order.
"""

import numpy as np
import ml_dtypes

import concourse.bacc as bacc
import concourse.mybir as mybir
import concourse.tile as tile
from concourse.bass_utils import run_bass_kernel_spmd

NCORES = 8
BF16 = ml_dtypes.bfloat16

# set by test harness to capture profiles
TRACE = False
TRACE_DIR = "/tmp/cwct_trace"
LAST_NS = {}
# overlap phase-2's NEFF compile (background thread + dummy run) with phase 1
PRECOMPILE_WARM = True


def _round_up(x, m):
    return (int(x) + m - 1) // m * m


def _p1_groups(T1):
    """Phase-1 DMA group tile counts per (feature, label): >=2 groups for
    pipelining, capped so a group tile stays under the SBUF budget."""
    ngroups = max(2, -(-T1 // 24))
    kts = []
    rem = T1
    for gi in range(ngroups):
        kt = -(-rem // (ngroups - gi))
        kts.append(kt)
        rem -= kt
    return kts


def _build_phase1_real(L, C, N):
    """Inputs gc/gs: (L, LBLK) bf16, host-swizzled pixel-major gathered
    tiles (+ones column); per label, _p1_groups(T1) DMA groups each laid
    out (128, KT, N+1) so one DMA pulls KT*(N+1)*2 contiguous bytes per
    SBUF partition.
    Outputs sc/ss: (L, 128, 386) f32 per label row block:
    [:, 0:256]   = S[0:128, 0:256] (upper row block, all columns)
    [:, 256]     = channel sums for channels 0..127
    [:, 257:385] = S[128:256, 128:256] (lower-right block)
    [:, 385]     = channel sums for channels 128..255
    (S[128:256, 0:128] is recovered on the host as S[0:128,128:256].T)"""
    assert N == 256
    T1 = C // 128
    KTS = _p1_groups(T1)
    W = 2 * (N + 1) - 128  # 386
    LBLK = T1 * 128 * (N + 1)
    nc = bacc.Bacc("TRN2", target_bir_lowering=False, debug=False, num_devices=NCORES)
    gc = nc.dram_tensor("gc", [L, LBLK], mybir.dt.bfloat16, kind="ExternalInput")
    gs = nc.dram_tensor("gs", [L, LBLK], mybir.dt.bfloat16, kind="ExternalInput")
    sc = nc.dram_tensor("sc", [L, 128, W], mybir.dt.float32, kind="ExternalOutput")
    ss = nc.dram_tensor("ss", [L, 128, W], mybir.dt.float32, kind="ExternalOutput")

    with tile.TileContext(nc) as tc:
        with (
            tc.tile_pool(name="gin", bufs=10) as gin,
            tc.tile_pool(name="out", bufs=4) as outp,
            tc.tile_pool(name="ps", bufs=8, space="PSUM") as psum,
        ):
            for g_dram, o_dram, ineng in ((gc, sc, nc.sync), (gs, ss, nc.sync)):
                for l in range(L):
                    ps0 = psum.tile([128, N + 1], mybir.dt.float32, tag="ps")
                    ps1 = psum.tile([128, 129], mybir.dt.float32, tag="ps")
                    n = 0
                    off = 0
                    for KT in KTS:
                        t = gin.tile([128, KTS[0], N + 1], mybir.dt.bfloat16, tag="g")
                        src = g_dram[l, off : off + 128 * KT * (N + 1)].rearrange(
                            "(p t c) -> p t c", p=128, t=KT, c=N + 1
                        )
                        ineng.dma_start(t[:, 0:KT, :], src)
                        off += 128 * KT * (N + 1)
                        for k in range(KT):
                            nc.tensor.matmul(
                                ps0[:], t[:, k, 0:128], t[:, k, :],
                                start=(n == 0), stop=(n == T1 - 1),
                            )
                            nc.tensor.matmul(
                                ps1[:], t[:, k, 128:256], t[:, k, 128 : N + 1],
                                start=(n == 0), stop=(n == T1 - 1),
                            )
                            n += 1
                    ob = outp.tile([128, W], mybir.dt.float32, tag="o")
                    nc.vector.tensor_copy(ob[:, 0 : N + 1], ps0[:])
                    nc.vector.tensor_copy(ob[:, N + 1 : W], ps1[:])
                    # scalar HWDGE ring: keep the sync ring free for inputs
                    nc.scalar.dma_start(o_dram[l], ob[:])
    nc.compile()
    return nc


def _build_phase2(L, C, N):
    """g2: (N, L*C) bf16 channel-major gathered content.
    tq: (128, L, 2, 2, 128) bf16 with tq[k,l,j,i,m] = T_l[i*128+m, j*128+k].
    bi: (128, 2, L) f32 with bi[p,i,l] = b_l[i*128+p].
    oc: (N, L*C) bf16 colored output (channel-major, gathered order)."""
    assert N == 256
    P2 = L * C
    assert C % 128 == 0

    nc = bacc.Bacc("TRN2", target_bir_lowering=False, debug=False, num_devices=NCORES)
    g2 = nc.dram_tensor("g2", [N, P2], mybir.dt.bfloat16, kind="ExternalInput")
    tq = nc.dram_tensor("tq", [128, L, 2, 2, 128], mybir.dt.bfloat16, kind="ExternalInput")
    bi = nc.dram_tensor("bi", [128, 2, L], mybir.dt.float32, kind="ExternalInput")
    oc = nc.dram_tensor("oc", [N, P2], mybir.dt.bfloat16, kind="ExternalOutput")

    with tile.TileContext(nc) as tc:
        with (
            tc.tile_pool(name="const", bufs=1) as constp,
            tc.tile_pool(name="gin", bufs=8) as gin,
            tc.tile_pool(name="out", bufs=8) as outp,
            tc.tile_pool(name="ps", bufs=4, space="PSUM") as psum,
        ):
            # constants on the scalar ring so the first pixel-block DMA is
            # not queued behind them on the sync ring
            tqt = constp.tile([128, L, 2, 2, 128], mybir.dt.bfloat16)
            nc.scalar.dma_start(tqt[:], tq[:])
            bit = constp.tile([128, 2, L], mybir.dt.float32)
            nc.scalar.dma_start(bit[:], bi[:])

            g2r = g2[:].rearrange("(j k) x -> k j x", j=2)
            ocr = oc[:].rearrange("(i k) x -> i k x", i=2)
            # groups of up to 1024 px per DMA, balanced so no group gets a
            # tiny DMA chunk; PSUM-bank-limited sub-blocks of <=512 px per
            # matmul
            ngrp = -(-C // 1024)
            gsz = []
            rem = C
            for gi in range(ngrp):
                g = -(-(rem // (ngrp - gi)) // 128) * 128
                gsz.append(g)
                rem -= g
            groups = []
            off = 0
            for g in gsz:
                subs = []
                so = 0
                while so < g:
                    s = min(512, g - so)
                    subs.append((so, s))
                    so += s
                groups.append((off, g, subs))
                off += g
            ocr2 = oc[:].rearrange("(i k) x -> k i x", i=2)
            for l in range(L):
                for off, G, subs in groups:
                    gt = gin.tile([128, 2, 1024], mybir.dt.bfloat16, tag="g")
                    nc.sync.dma_start(
                        gt[:, :, 0:G], g2r[:, :, l * C + off : l * C + off + G]
                    )
                    # both i-chunks evict into one tile -> a single output
                    # DMA per group (halves the DMA-issue load on ACT)
                    ob = outp.tile([128, 2, 1024], mybir.dt.bfloat16, tag="o")
                    for i in range(2):
                        # one 2-bank PSUM region per (group, i); each <=512
                        # sub-block's matmuls stay within one bank
                        ps = psum.tile([128, 1024], mybir.dt.float32, tag="ps")
                        for so, S in subs:
                            nc.tensor.matmul(
                                ps[:, so : so + S], tqt[:, l, 0, i, :],
                                gt[:, 0, so : so + S], start=True, stop=False,
                            )
                            nc.tensor.matmul(
                                ps[:, so : so + S], tqt[:, l, 1, i, :],
                                gt[:, 1, so : so + S], start=False, stop=True,
                            )
                        # evictions split across the two elementwise engines
                        # so neither stalls PSUM recycling
                        if i == 0:
                            nc.vector.tensor_scalar_add(
                                ob[:, 0, 0:G], ps[:, 0:G], bit[:, i, l : l + 1]
                            )
                        else:
                            nc.scalar.activation(
                                ob[:, 1, 0:G], ps[:, 0:G],
                                mybir.ActivationFunctionType.Identity,
                                bias=bit[:, i, l : l + 1],
                            )
                    nc.scalar.dma_start(
                        ocr2[:, :, l * C + off : l * C + off + G], ob[:, :, 0:G]
                    )
    nc.compile()
    return nc


def _run(nc, in_maps, label):
    if TRACE:
        import os
        import shutil

        tdir = f"{TRACE_DIR}/{label}"
        shutil.rmtree(tdir, ignore_errors=True)
        os.makedirs(tdir, exist_ok=True)
        res = run_bass_kernel_spmd(
            nc, in_maps, list(range(NCORES)), trace=True, tmpdir=tdir
        )
        LAST_NS[label] = res.exec_time_ns
    else:
        res = run_bass_kernel_spmd(nc, in_maps, list(range(NCORES)))
    return res


def kernel(content_feat, style_feat, content_seg, style_seg, num_labels):
    L = int(num_labels)
    B, N, H, W = content_feat.shape
    M = H * W
    assert B == 1 and N == 256

    c = np.asarray(content_feat, dtype=np.float32).reshape(N, M)
    s = np.asarray(style_feat, dtype=np.float32).reshape(N, M)
    seg_c = np.asarray(content_seg).reshape(M).astype(np.int64)
    seg_s = np.asarray(style_seg).reshape(M).astype(np.int64)

    order_c = np.argsort(seg_c, kind="stable")
    order_s = np.argsort(seg_s, kind="stable")
    counts_c = np.bincount(seg_c, minlength=L)[:L]
    counts_s = np.bincount(seg_s, minlength=L)[:L]

    def split_counts(cnt):
        base = cnt // NCORES
        out = np.tile(base[:, None], (1, NCORES))
        for l in range(L):
            out[l, : cnt[l] % NCORES] += 1
        return out

    cc = split_counts(counts_c)  # (L, NCORES)
    cs = split_counts(counts_s)

    C = _round_up(max(cc.max(), cs.max()), 128)
    P = L * C

    cT_bf = np.ascontiguousarray(c.T).astype(BF16)  # (M, N)
    sT_bf = np.ascontiguousarray(s.T).astype(BF16)

    def build_gathers(xT, order, counts, core_counts):
        lab_pos = np.concatenate(([0], np.cumsum(counts)))
        arrs = [np.zeros((P, N + 1), dtype=BF16) for _ in range(NCORES)]
        for l in range(L):
            off = lab_pos[l]
            for k in range(NCORES):
                m = int(core_counts[l, k])
                if m:
                    a = arrs[k]
                    a[l * C : l * C + m, :N] = xT[order[off : off + m]]
                    a[l * C : l * C + m, N] = 1.0
                off += m
        return arrs

    gc_arrs = build_gathers(cT_bf, order_c, counts_c, cc)
    gs_arrs = build_gathers(sT_bf, order_s, counts_s, cs)
    del sT_bf

    # kick off phase-2 build + a dummy warm-up run in the background so its
    # NEFF compile overlaps phase 1's (wall-clock only; device results of the
    # dummy run are discarded). Falls back to the serial path on any failure.
    p2_box = {}

    def _precompile_p2():
        try:
            nc2 = _build_phase2(L, C, N)
            if PRECOMPILE_WARM:
                z = {
                    "g2": np.zeros((N, L * C), dtype=BF16),
                    "tq": np.zeros((128, L, 2, 2, 128), dtype=BF16),
                    "bi": np.zeros((128, 2, L), dtype=np.float32),
                }
                run_bass_kernel_spmd(nc2, [z] * NCORES, list(range(NCORES)))
            p2_box["nc"] = nc2
        except Exception as e:  # pragma: no cover - fallback path
            p2_box["err"] = e

    import threading

    p2_thread = threading.Thread(target=_precompile_p2, daemon=True)
    p2_thread.start()

    # swizzle for phase 1: per label, DMA groups of tiles, each group laid
    # out (128, KT, N+1) so DMA chunks are contiguous per SBUF partition
    T1 = C // 128
    KTS = _p1_groups(T1)

    def swizzle(a):
        tiles = a.reshape(L, T1, 128, N + 1)
        out = np.empty((L, T1 * 128 * (N + 1)), dtype=a.dtype)
        for l in range(L):
            pos = 0
            t0 = 0
            for kt in KTS:
                n = kt * 128 * (N + 1)
                out[l, pos : pos + n] = tiles[l, t0 : t0 + kt].transpose(1, 0, 2).reshape(-1)
                pos += n
                t0 += kt
        return out

    nc1p = _build_phase1_real(L, C, N)
    if TRACE:
        # keep the traced phase-1 profile free of the background warm-up run
        p2_thread.join()
    res1 = _run(
        nc1p,
        [{"gc": swizzle(gc_arrs[k]), "gs": swizzle(gs_arrs[k])} for k in range(NCORES)],
        "p1",
    )

    # host: all-reduce moments, finish stats, cholesky, transforms (float64)
    PW = 2 * (N + 1) - 128
    sc_sum = np.zeros((L, 128, PW), dtype=np.float64)
    ss_sum = np.zeros((L, 128, PW), dtype=np.float64)
    for k in range(NCORES):
        sc_sum += res1.results[k]["sc"]
        ss_sum += res1.results[k]["ss"]

    def unpack(ssum, l):
        Sm = np.empty((N, N), dtype=np.float64)
        Sm[0:128, :] = ssum[l, :, 0:N]
        Sm[128:N, 128:N] = ssum[l, :, N + 1 : N + 129]
        Sm[128:N, 0:128] = Sm[0:128, 128:N].T
        sums = np.concatenate([ssum[l, :, N], ssum[l, :, PW - 1]], axis=0)
        return Sm, sums

    eyeN = np.eye(N, dtype=np.float64)
    T_all = np.zeros((L, N, N), dtype=np.float64)
    b_all = np.zeros((L, N), dtype=np.float64)
    valid = np.zeros(L, dtype=bool)

    try:
        from scipy.linalg import solve_triangular as _st

        def tri_inv(Lm):
            return _st(Lm, eyeN, lower=True)
    except ImportError:

        def tri_inv(Lm):
            return np.linalg.solve(Lm, eyeN)

    for l in range(L):
        ncnt = float(counts_c[l])
        nsnt = float(counts_s[l])
        v = (ncnt > 10) and (nsnt > 10) and (ncnt < 100.0 * nsnt) and (nsnt < 100.0 * ncnt)
        Tl, bl = eyeN, np.zeros(N)
        if v:
            Sc, sum_c = unpack(sc_sum, l)
            Ss, sum_s = unpack(ss_sum, l)
            mc = sum_c / max(ncnt, 1.0)
            ms = sum_s / max(nsnt, 1.0)
            cov_c = (Sc - ncnt * np.outer(mc, mc)) / max(max(ncnt, 1.0) - 1.0, 1.0)
            cov_s = (Ss - nsnt * np.outer(ms, ms)) / max(max(nsnt, 1.0) - 1.0, 1.0)
            try:
                Lc = np.linalg.cholesky(cov_c)
                Ls = np.linalg.cholesky(cov_s)
                Tl = Ls @ tri_inv(Lc)
                bl = ms - Tl @ mc
            except np.linalg.LinAlgError:
                v, Tl, bl = False, eyeN, np.zeros(N)
        T_all[l], b_all[l], valid[l] = Tl, bl, v

    # phase-2 inputs
    tq_np = np.zeros((128, L, 2, 2, 128), dtype=BF16)
    for l in range(L):
        Tl = T_all[l].astype(np.float32)
        for j in range(2):
            for i in range(2):
                tq_np[:, l, j, i, :] = Tl[
                    i * 128 : (i + 1) * 128, j * 128 : (j + 1) * 128
                ].T
    bi_np = np.zeros((128, 2, L), dtype=np.float32)
    for l in range(L):
        for i in range(2):
            bi_np[:, i, l] = b_all[l][i * 128 : (i + 1) * 128]

    g2_arrs = [np.ascontiguousarray(gc_arrs[k][:, :N].T) for k in range(NCORES)]

    p2_thread.join()
    nc2p = p2_box.get("nc")
    if nc2p is None:
        nc2p = _build_phase2(L, C, N)
    res2 = _run(
        nc2p,
        [{"g2": g2_arrs[k], "tq": tq_np, "bi": bi_np} for k in range(NCORES)],
        "p2",
    )

    # assemble: gathered order -> sorted order -> original pixel order
    cT32 = None
    sorted_pm = np.empty((M, N), dtype=np.float32)
    pos = 0
    for l in range(L):
        for k in range(NCORES):
            m = int(cc[l, k])
            if m:
                if valid[l]:
                    sorted_pm[pos : pos + m] = np.asarray(
                        res2.results[k]["oc"].T[l * C : l * C + m], dtype=np.float32
                    )
                else:
                    if cT32 is None:
                        cT32 = np.ascontiguousarray(c.T)
                    sorted_pm[pos : pos + m] = cT32[order_c[pos : pos + m]]
            pos += m

    # pixels whose label is outside [0, L) are untouched by the reference
    if pos < M:
        if cT32 is None:
            cT32 = np.ascontiguousarray(c.T)
        sorted_pm[pos:] = cT32[order_c[pos:]]

    final_pm = np.empty((M, N), dtype=np.float32)
    final_pm[order_c] = sorted_pm
    return np.ascontiguousarray(final_pm.T).reshape(B, N, H, W)
